# revision 24
# baseline (speedup 1.0000x reference)
"""BGRL posgraph kernel for 8 Trainium2 NeuronCores.

Computes, given online_x/target_x [16384,512] and weights [512,512]:
  online_y = online_x @ W_online
  target_y = target_x @ W_target
  knn      = top-8 teacher indices per student row of
             cosine_sim(l2norm(online_y), l2norm(target_y))
returns (online_y f32 [16384,512], target_y f32 [16384,512], knn int32 [2,131072]).

Sharding: student rows split 8 ways (2048 rows/core). The normalized teacher
matrix is built once in a transposed blocked layout in HBM scratch — computed
1/8th per core and AllGathered across the 8 cores ("b3cc"/"f32cc" modes, the
default), or recomputed fully per core ("b3"/"f32", the automatic fallback if
the collective path fails). Each core then streams the full teacher against
its own student rows, producing [2048, 16384] similarity blocks chunk by
chunk, and takes a per-2048-column-segment top-8 with the DVE Max8/MaxIndex
instructions (64 candidates per row). The host rescores those candidates in
fp32 and keeps the top 8, ordered like jax.lax.top_k (stable by (-value,
index)).

Precision:
 - encoder matmuls + teacher normalization: native fp32 PE matmuls (~2e-7 rel)
 - similarity: "b3*" splits both operands into bf16 hi+lo and computes 3 of
   the 4 cross products (~2e-6 rel, 3 PE cycles/row vs fp32's 4), "f32*" uses
   native fp32. Per-2048-column candidate sets are insensitive to this noise;
   the final ordering is fixed by the host rescore.
 - student rows are NOT l2-normalized on device: a per-row positive scale
   cannot change that row's top-k.

Per-core modeled time (CoreSim cost-model timeline): 1.75 ms — TensorE-bound
at ~97% occupancy (1.31 ms is the 3-pass bf16 similarity matmul roofline,
~0.14 ms encoders/transposes, ~0.3 ms modeled AllGather exposure).
"""

import os
import sys

sys.path.insert(0, "/opt/trn_rl_repo")

import numpy as np
from contextlib import ExitStack

import concourse.bass as bass
import concourse.tile as tile
import concourse.mybir as mybir
from concourse import bacc, masks
from concourse.bass_utils import run_bass_kernel_spmd

F32 = mybir.dt.float32
BF16 = mybir.dt.bfloat16
U32 = mybir.dt.uint32
AF = mybir.ActivationFunctionType

N = 16384            # nodes
D = 512              # feature dim
NCORES = 8
RPC = N // NCORES    # rows per core (2048)
ST = RPC // 128      # student tiles per core (16)
TT = N // 128        # teacher tiles (128)
KT = D // 128        # contraction tiles (4)
CH = 512             # sim chunk columns
NCHUNK = N // CH     # 32
SEG = 2048           # top-k segment width
NSEG = N // SEG      # 8
CPS = SEG // CH      # chunks per segment (4)
G = 8                # student tiles per teacher sweep
NSWEEP = ST // G     # 2

MODE = os.environ.get("BGRL_SIM_MODE", "b3cc")  # b3 | f32 | b3cc | f32cc

_CACHE = {}

# exec time of the last traced run (ns), populated when BGRL_TRACE=1
LAST_EXEC_NS = None
LAST_RESULTS = None


def _transpose_512(nc, pool_ps, act_dst, src_ap, ident, tag="ps_t"):
    """PE-transpose src [128, 512] -> dst tile [128, 4, 128] (d-chunk, row)."""
    ps = pool_ps.tile([128, 512], F32, tag=tag, name=tag)
    for k in range(KT):
        nc.tensor.transpose(ps[:, bass.ts(k, 128)], src_ap[:, bass.ts(k, 128)], ident)
    nc.scalar.copy(act_dst, ps[:].rearrange("p (a b) -> p a b", a=KT))


def build(mode):
    b3 = mode.startswith("b3")
    cc = mode.endswith("cc")
    # timing-only hack: emit the cc-structured program without the collective
    cc_emit = cc and not os.environ.get("BGRL_SKIP_CC")
    nc = bacc.Bacc("TRN2", target_bir_lowering=False, debug=False,
                   num_devices=NCORES)

    ox = nc.dram_tensor("online_x_own", [RPC, D], F32, kind="ExternalInput").ap()
    tx_own = nc.dram_tensor("target_x_own", [RPC, D], F32, kind="ExternalInput").ap()
    if not cc:
        tx_full = nc.dram_tensor("target_x_full", [N, D], F32, kind="ExternalInput").ap()
    w_on = nc.dram_tensor("W_online", [D, D], F32, kind="ExternalInput").ap()
    w_tg = nc.dram_tensor("W_target", [D, D], F32, kind="ExternalInput").ap()

    oy = nc.dram_tensor("online_y", [RPC, D], F32, kind="ExternalOutput").ap()
    ty = nc.dram_tensor("target_y", [RPC, D], F32, kind="ExternalOutput").ap()
    cv = nc.dram_tensor("cand_v", [RPC, NSEG, 8], F32, kind="ExternalOutput").ap()
    ci = nc.dram_tensor("cand_i", [RPC, NSEG, 8], U32, kind="ExternalOutput").ap()

    # teacher scratch: blocked transposed layout [tt, partition, k, col]
    sdt = BF16 if b3 else F32
    n_sc = 2 if b3 else 1
    sc_space = "Shared" if cc_emit else "Local"
    NGATH = int(os.environ.get("BGRL_NGATH", "1")) if cc else 1
    TPG = ST // NGATH  # teacher tiles per sub-gather
    if cc:
        # per-sub-gather scratch holding hi+lo together (one AllGather moves
        # both): scr[g][r*TPG + t, i] = split i of teacher tile r*ST + g*TPG + t
        scr = [nc.dram_tensor(f"tsc_{g}", [NCORES * TPG, n_sc, 128, KT, 128],
                              sdt, kind="Internal", addr_space=sc_space).ap()
               for g in range(NGATH)]
        loc = nc.dram_tensor("loc", [ST, n_sc, 128, KT, 128], sdt,
                             kind="Internal").ap()

        def scr_block(i, tt):
            g = (tt % ST) // TPG
            pos = (tt // ST) * TPG + (tt % TPG)
            return scr[g][pos, i]

        def loc_block(i, t):
            return loc[t, i]
    else:
        scr0 = nc.dram_tensor("tsc", [TT, n_sc, 128, KT, 128], sdt,
                              kind="Internal").ap()

        def scr_block(i, tt):
            return scr0[tt, i]

    with tile.TileContext(nc) as tc, ExitStack() as ctx:
        const = ctx.enter_context(tc.tile_pool(name="const", bufs=1))
        psum = ctx.enter_context(tc.tile_pool(name="psum", bufs=2, space="PSUM"))

        ident = const.tile([128, 128], F32, tag="ident")
        masks.make_identity(nc, ident[:])

        wo_sb = const.tile([128, KT, D], F32, tag="wo")
        wt_sb = const.tile([128, KT, D], F32, tag="wt")
        nc.sync.dma_start(wo_sb[:], w_on.rearrange("(k p) n -> p k n", p=128))
        nc.sync.dma_start(wt_sb[:], w_tg.rearrange("(k p) n -> p k n", p=128))

        # student stationary operands for the similarity matmul
        if b3:
            yt_hi = const.tile([128, KT, RPC], BF16, tag="yt_hi")
            yt_lo = const.tile([128, KT, RPC], BF16, tag="yt_lo")
        else:
            yt_f = const.tile([128, KT, RPC], F32, tag="yt_f")

        def ps_tile(tag):
            return psum.tile([128, 512], F32, tag=tag, name=tag)

        def encode_tile(ab, x_rows, w_sb):
            """x rows [128, D] --DMA+transpose+matmul--> psum tile holding y."""
            xt_in = ab.tile([128, D], F32, tag="x_in")
            nc.sync.dma_start(xt_in[:], x_rows)
            xT = ab.tile([128, KT, 128], F32, tag="xT")
            _transpose_512(nc, psum, xT[:], xt_in[:], ident[:], tag="ps_tx")
            psy = ps_tile("ps_y")
            for k in range(KT):
                nc.tensor.matmul(psy[:], xT[:, k, :], w_sb[:, k, :],
                                 start=(k == 0), stop=(k == KT - 1))
            return psy

        def teacher_norm_tile(ab, psy, dest_fn):
            """psum y -> normalized+transposed (+hi/lo split) -> dest_fn(i) DRAM."""
            sq = ab.tile([128, D], F32, tag="sq")
            n2 = ab.tile([128, 1], F32, tag="n2")
            nc.scalar.activation(sq[:], psy[:], AF.Square, accum_out=n2[:])
            inv = ab.tile([128, 1], F32, tag="inv")
            nc.vector.reciprocal(inv[:], n2[:])
            rn = ab.tile([128, 1], F32, tag="rn")
            nc.scalar.activation(rn[:], inv[:], AF.Sqrt)
            tn = ab.tile([128, D], F32, tag="tn")
            nc.vector.tensor_scalar_mul(tn[:], psy[:], rn[:])
            tnT = ab.tile([128, KT, 128], F32, tag="tnT")
            _transpose_512(nc, psum, tnT[:], tn[:], ident[:], tag="ps_tn")
            if b3:
                tnh = ab.tile([128, KT, 128], BF16, tag="tnh")
                tnl = ab.tile([128, KT, 128], BF16, tag="tnl")
                tmp = ab.tile([128, KT, 128], F32, tag="tmp")
                nc.vector.tensor_copy(tnh[:], tnT[:])
                nc.vector.tensor_sub(tmp[:], tnT[:], tnh[:])
                nc.vector.tensor_copy(tnl[:], tmp[:])
                nc.sync.dma_start(dest_fn(0), tnh[:])
                nc.sync.dma_start(dest_fn(1), tnl[:])
            else:
                nc.sync.dma_start(dest_fn(0), tnT[:])

        def emit_phase_ab(ab):
            if cc:
                # Phase B_own: normalize own teacher slice + emit target_y.
                # 1-tile software pipeline: emit tile t+1's encode (PE work)
                # before tile t's normalization chain (ACT/DVE latency) so the
                # PE never waits on the norm chain.
                def finish_teacher(t, psy):
                    yb = ab.tile([128, D], F32, tag="ya", name="yb")
                    nc.scalar.copy(yb[:], psy[:])
                    nc.sync.dma_start(ty[bass.ts(t, 128), :], yb[:])
                    teacher_norm_tile(ab, psy, lambda i, t=t: loc_block(i, t))

                prev = None
                for t in range(ST):
                    psy = encode_tile(ab, tx_own[bass.ts(t, 128), :], wt_sb)
                    if prev is not None:
                        finish_teacher(*prev)
                    prev = (t, psy)
                finish_teacher(*prev)
                if cc_emit:
                    for gth in range(NGATH):
                        nc.gpsimd.collective_compute(
                            "AllGather",
                            mybir.AluOpType.bypass,
                            replica_groups=[list(range(NCORES))],
                            ins=[loc[bass.ts(gth, TPG)]],
                            outs=[scr[gth]],
                        )
            else:
                # Phase B: recompute the full teacher on every core
                for tt in range(TT):
                    psy = encode_tile(ab, tx_full[bass.ts(tt, 128), :], wt_sb)
                    teacher_norm_tile(ab, psy, lambda i, tt=tt: scr_block(i, tt))
                # Phase B': target own rows -> target_y output
                for t in range(ST):
                    psy = encode_tile(ab, tx_own[bass.ts(t, 128), :], wt_sb)
                    yb = ab.tile([128, D], F32, tag="ya", name="yb")
                    nc.scalar.copy(yb[:], psy[:])
                    nc.sync.dma_start(ty[bass.ts(t, 128), :], yb[:])

            # Phase A: student own rows -> online_y + yT stationary
            for t in range(ST):
                psy = encode_tile(ab, ox[bass.ts(t, 128), :], wo_sb)
                ya = ab.tile([128, D], F32, tag="ya", name="ya")
                nc.scalar.copy(ya[:], psy[:])
                nc.sync.dma_start(oy[bass.ts(t, 128), :], ya[:])
                yT = ab.tile([128, KT, 128], F32, tag="yT", name="yT")
                _transpose_512(nc, psum, yT[:], ya[:], ident[:], tag="ps_tn")
                if b3:
                    nc.vector.tensor_copy(yt_hi[:, :, bass.ts(t, 128)], yT[:])
                    tmp2 = ab.tile([128, KT, 128], F32, tag="tmp2", name="tmp2")
                    nc.vector.tensor_sub(tmp2[:], yT[:], yt_hi[:, :, bass.ts(t, 128)])
                    nc.vector.tensor_copy(yt_lo[:, :, bass.ts(t, 128)], tmp2[:])
                else:
                    nc.vector.tensor_copy(yt_f[:, :, bass.ts(t, 128)], yT[:])

        def emit_phase_c(cstream, csegs, ccand):
            _phase_c_body(cstream, csegs, ccand)

        def _phase_c_body(cstream, csegs, ccand):
            for sweep in range(NSWEEP):
                _phase_c_sweep(cstream, csegs, ccand, sweep)

        loop_n = int(os.environ.get("BGRL_LOOP", "0"))
        trunc = int(os.environ.get("BGRL_TRUNC", "0"))

        def _phase_c_sweep(cstream, csegs, ccand, sweep):
            segs = [csegs.tile([128, SEG], F32, tag=f"seg{st}", name=f"seg{st}")
                    for st in range(G)]
            cvs = [ccand.tile([128, NSEG, 8], F32, tag=f"cv{st}", name=f"cv{st}")
                   for st in range(G)]
            cis = [ccand.tile([128, NSEG, 8], U32, tag=f"ci{st}", name=f"ci{st}")
                   for st in range(G)]
            for c in range(trunc if trunc else NCHUNK):
                tstr = []
                for i in range(n_sc):
                    tsb = cstream.tile([128, KT, 4, 128], sdt, tag=f"ts{i}",
                                       name=f"ts{i}")
                    for j in range(4):
                        nc.sync.dma_start(tsb[:, :, j, :], scr_block(i, 4 * c + j))
                    tstr.append(tsb)
                for st in range(G):
                    g = sweep * G + st
                    ps = ps_tile("ps_c")
                    if b3:
                        th, tl = tstr
                        n_mm = 3 * KT
                        i = 0
                        for k in range(KT):
                            s_hi = yt_hi[:, k, bass.ts(g, 128)]
                            s_lo = yt_lo[:, k, bass.ts(g, 128)]
                            for lhs, rhs in ((s_hi, th), (s_hi, tl), (s_lo, th)):
                                nc.tensor.matmul(ps[:], lhs, rhs[:, k, :, :],
                                                 start=(i == 0), stop=(i == n_mm - 1))
                                i += 1
                    else:
                        for k in range(KT):
                            nc.tensor.matmul(ps[:], yt_f[:, k, bass.ts(g, 128)],
                                             tstr[0][:, k, :, :],
                                             start=(k == 0), stop=(k == KT - 1))
                    nc.scalar.copy(segs[st][:, bass.ts(c % CPS, CH)], ps[:])
                if c % CPS == CPS - 1:
                    si = c // CPS
                    for st in range(G):
                        nc.vector.max(cvs[st][:, si, :], segs[st][:])
                        nc.vector.max_index(cis[st][:, si, :], cvs[st][:, si, :],
                                            segs[st][:])
            for st in range(G):
                g = sweep * G + st
                nc.sync.dma_start(cv[bass.ts(g, 128)], cvs[st][:])
                nc.sync.dma_start(ci[bass.ts(g, 128)], cis[st][:])

        if loop_n:
            # timing mode: run the whole kernel body in a hardware loop
            ab = ctx.enter_context(tc.tile_pool(name="ab", bufs=2))
            cstream = ctx.enter_context(tc.tile_pool(name="cstream", bufs=3))
            csegs = ctx.enter_context(tc.tile_pool(name="csegs", bufs=1))
            ccand = ctx.enter_context(tc.tile_pool(name="ccand", bufs=2))
            with tc.For_i(0, loop_n, 1):
                emit_phase_ab(ab)
                emit_phase_c(cstream, csegs, ccand)
        else:
            with tc.tile_pool(name="ab", bufs=3) as ab:
                emit_phase_ab(ab)
            cstream = ctx.enter_context(tc.tile_pool(name="cstream", bufs=3))
            csegs = ctx.enter_context(tc.tile_pool(name="csegs", bufs=1))
            ccand = ctx.enter_context(tc.tile_pool(name="ccand", bufs=2))
            emit_phase_c(cstream, csegs, ccand)

    nc.compile()
    return nc


def _get_nc(mode):
    if mode not in _CACHE:
        _CACHE[mode] = build(mode)
    return _CACHE[mode]


def _host_topk_rows(online_y, target_y, rows):
    """Exact host recompute of top-8 teacher indices for the given rows."""
    tn = target_y / np.maximum(
        np.sqrt((target_y.astype(np.float64) ** 2).sum(-1, keepdims=True)), 1e-12)
    s = online_y[rows].astype(np.float64)
    sim = s @ tn.T.astype(np.float64)
    return np.argsort(-sim, axis=1, kind="stable")[:, :8]


def kernel(online_x, target_x, W_online, W_pred, W_target, k):
    global LAST_EXEC_NS, LAST_RESULTS
    assert int(k) == 8
    online_x = np.ascontiguousarray(np.asarray(online_x, dtype=np.float32))
    target_x = np.ascontiguousarray(np.asarray(target_x, dtype=np.float32))
    W_online = np.ascontiguousarray(np.asarray(W_online, dtype=np.float32))
    W_target = np.ascontiguousarray(np.asarray(W_target, dtype=np.float32))

    def _run(mode):
        nc = _get_nc(mode)
        cc = mode.endswith("cc")
        in_maps = []
        for c in range(NCORES):
            sl = slice(c * RPC, (c + 1) * RPC)
            m = {
                "online_x_own": np.ascontiguousarray(online_x[sl]),
                "target_x_own": np.ascontiguousarray(target_x[sl]),
                "W_online": W_online,
                "W_target": W_target,
            }
            if not cc:
                m["target_x_full"] = target_x
            in_maps.append(m)
        trace = os.environ.get("BGRL_TRACE", "0") == "1"
        return run_bass_kernel_spmd(nc, in_maps, core_ids=list(range(NCORES)),
                                    trace=trace)

    try:
        res = _run(MODE)
    except Exception:
        if not MODE.endswith("cc"):
            raise
        # collective path failed in this environment: fall back to the
        # variant that recomputes the full teacher on every core
        res = _run(MODE[:-2])
    LAST_EXEC_NS = res.exec_time_ns
    LAST_RESULTS = res

    outs = res.results
    online_y = np.concatenate([outs[c]["online_y"] for c in range(NCORES)], axis=0)
    target_y = np.concatenate([outs[c]["target_y"] for c in range(NCORES)], axis=0)
    vals = np.concatenate([outs[c]["cand_v"] for c in range(NCORES)], axis=0)
    idxs = np.concatenate([outs[c]["cand_i"] for c in range(NCORES)], axis=0)

    # merge the NSEG per-segment candidate lists
    gi = idxs.astype(np.int64) + (np.arange(NSEG, dtype=np.int64) * SEG)[None, :, None]
    flat_i = gi.reshape(N, NSEG * 8)

    # defensive: MaxIndex emits 0xFFFFFFFF when a value is unmatched; clamp those
    # candidates to a valid index (their value row is recomputed below anyway)
    bad_mask = (flat_i < 0) | (flat_i >= N)
    bad_rows = np.unique(np.nonzero(bad_mask)[0])
    flat_i[bad_mask] = 0

    if os.environ.get("BGRL_RESCORE", "1") == "1":
        # rescore the 64 candidates per row on host in fp32 (cheap, exact
        # ordering match with the fp32 reference up to its own rounding)
        def l2n(x):
            nn = np.sqrt((x * x).sum(-1, keepdims=True))
            return x / np.maximum(nn, 1e-12)

        student = l2n(online_y)
        teacher = l2n(target_y)
        top_idx = np.zeros((N, 8), dtype=np.int64)
        B = 2048
        for i in range(0, N, B):
            cand = np.sort(flat_i[i:i + B], axis=1)      # index-ascending
            tvec = teacher[cand]                          # [B, 64, 512]
            sims = np.matmul(tvec, student[i:i + B, :, None])[:, :, 0]  # [B, 64]
            o2 = np.argsort(-sims, axis=1, kind="stable")[:, :8]
            top_idx[i:i + B] = np.take_along_axis(cand, o2, axis=1)
    else:
        flat_v = vals.reshape(N, NSEG * 8)
        order = np.argsort(-flat_v, axis=1, kind="stable")[:, :8]
        top_idx = np.take_along_axis(flat_i, order, axis=1)

    if bad_rows.size:
        top_idx[bad_rows] = _host_topk_rows(online_y, target_y, bad_rows)

    rows = np.repeat(np.arange(N, dtype=np.int32), 8)
    cols = top_idx.reshape(-1).astype(np.int32)
    knn = np.stack([rows, cols], axis=0)
    return online_y, target_y, knn


# revision 25
# speedup vs baseline: 1.2832x; 1.2832x over previous
"""BGRL posgraph kernel for 8 Trainium2 NeuronCores.

Computes, given online_x/target_x [16384,512] and weights [512,512]:
  online_y = online_x @ W_online
  target_y = target_x @ W_target
  knn      = top-8 teacher indices per student row of
             cosine_sim(l2norm(online_y), l2norm(target_y))
returns (online_y f32 [16384,512], target_y f32 [16384,512], knn int32 [2,131072]).

Sharding: student rows split 8 ways (2048 rows/core). The normalized teacher
matrix is built once in a transposed blocked layout in HBM scratch — computed
1/8th per core and AllGathered across the 8 cores ("b3cc"/"f32cc" modes, the
default), or recomputed fully per core ("b3"/"f32", the automatic fallback if
the collective path fails). Each core then streams the full teacher against
its own student rows, producing [2048, 16384] similarity blocks chunk by
chunk, and takes a per-2048-column-segment top-8 with the DVE Max8/MaxIndex
instructions (64 candidates per row). The host rescores those candidates in
fp32 and keeps the top 8, ordered like jax.lax.top_k (stable by (-value,
index)).

Precision:
 - encoder matmuls + teacher normalization: native fp32 PE matmuls (~2e-7 rel)
 - similarity: "b3*" splits both operands into bf16 hi+lo and computes 3 of
   the 4 cross products (~2e-6 rel, 3 PE cycles/row vs fp32's 4), "f32*" uses
   native fp32. Per-2048-column candidate sets are insensitive to this noise;
   the final ordering is fixed by the host rescore.
 - student rows are NOT l2-normalized on device: a per-row positive scale
   cannot change that row's top-k.

Per-core modeled time (CoreSim cost-model timeline): 1.75 ms — TensorE-bound
at ~97% occupancy (1.31 ms is the 3-pass bf16 similarity matmul roofline,
~0.14 ms encoders/transposes, ~0.3 ms modeled AllGather exposure).
"""

import os
import sys

sys.path.insert(0, "/opt/trn_rl_repo")

import numpy as np
from contextlib import ExitStack

import concourse.bass as bass
import concourse.tile as tile
import concourse.mybir as mybir
from concourse import bacc, masks
from concourse.bass_utils import run_bass_kernel_spmd

F32 = mybir.dt.float32
BF16 = mybir.dt.bfloat16
F8E5 = mybir.dt.float8e5
U32 = mybir.dt.uint32
AF = mybir.ActivationFunctionType

N = 16384            # nodes
D = 512              # feature dim
NCORES = 8
RPC = N // NCORES    # rows per core (2048)
ST = RPC // 128      # student tiles per core (16)
TT = N // 128        # teacher tiles (128)
KT = D // 128        # contraction tiles (4)
CH = 512             # sim chunk columns
NCHUNK = N // CH     # 32
SEG = 2048           # top-k segment width
NSEG = N // SEG      # 8
CPS = SEG // CH      # chunks per segment (4)
G = 8                # student tiles per teacher sweep
NSWEEP = ST // G     # 2

MODE = os.environ.get("BGRL_SIM_MODE", "b3cc")  # b3 | f32 | b3cc | f32cc
# fp8e5m2 DoubleRow for the hi*lo cross terms (b3 modes only): all similarity
# terms are scaled by a uniform 16 (ranking-invariant; host rescore fixes values)
X8 = os.environ.get("BGRL_X8", "0") == "1"

_CACHE = {}

# exec time of the last traced run (ns), populated when BGRL_TRACE=1
LAST_EXEC_NS = None
LAST_RESULTS = None


def _transpose_512(nc, pool_ps, act_dst, src_ap, ident, tag="ps_t"):
    """PE-transpose src [128, 512] -> dst tile [128, 4, 128] (d-chunk, row)."""
    ps = pool_ps.tile([128, 512], F32, tag=tag, name=tag)
    for k in range(KT):
        nc.tensor.transpose(ps[:, bass.ts(k, 128)], src_ap[:, bass.ts(k, 128)], ident)
    nc.scalar.copy(act_dst, ps[:].rearrange("p (a b) -> p a b", a=KT))


def build(mode):
    b3 = mode.startswith("b3")
    cc = mode.endswith("cc")
    # timing-only hack: emit the cc-structured program without the collective
    cc_emit = cc and not os.environ.get("BGRL_SKIP_CC")
    nc = bacc.Bacc("TRN2", target_bir_lowering=False, debug=False,
                   num_devices=NCORES)

    ox = nc.dram_tensor("online_x_own", [RPC, D], F32, kind="ExternalInput").ap()
    tx_own = nc.dram_tensor("target_x_own", [RPC, D], F32, kind="ExternalInput").ap()
    if not cc:
        tx_full = nc.dram_tensor("target_x_full", [N, D], F32, kind="ExternalInput").ap()
    w_on = nc.dram_tensor("W_online", [D, D], F32, kind="ExternalInput").ap()
    w_tg = nc.dram_tensor("W_target", [D, D], F32, kind="ExternalInput").ap()

    oy = nc.dram_tensor("online_y", [RPC, D], F32, kind="ExternalOutput").ap()
    ty = nc.dram_tensor("target_y", [RPC, D], F32, kind="ExternalOutput").ap()
    cv = nc.dram_tensor("cand_v", [RPC, NSEG, 8], F32, kind="ExternalOutput").ap()
    ci = nc.dram_tensor("cand_i", [RPC, NSEG, 8], U32, kind="ExternalOutput").ap()

    # teacher scratch: blocked transposed layout [tt, partition, k, col]
    sdt = BF16 if b3 else F32
    n_sc = 2 if b3 else 1
    sc_space = "Shared" if cc_emit else "Local"
    NGATH = int(os.environ.get("BGRL_NGATH", "1")) if cc else 1
    TPG = ST // NGATH  # teacher tiles per sub-gather
    if cc:
        # per-sub-gather scratch holding hi+lo together (one AllGather moves
        # both): scr[g][r*TPG + t, i] = split i of teacher tile r*ST + g*TPG + t
        scr = [nc.dram_tensor(f"tsc_{g}", [NCORES * TPG, n_sc, 128, KT, 128],
                              sdt, kind="Internal", addr_space=sc_space).ap()
               for g in range(NGATH)]
        loc = nc.dram_tensor("loc", [ST, n_sc, 128, KT, 128], sdt,
                             kind="Internal").ap()

        def scr_block(i, tt):
            g = (tt % ST) // TPG
            pos = (tt // ST) * TPG + (tt % TPG)
            return scr[g][pos, i]

        def loc_block(i, t):
            return loc[t, i]
    else:
        scr0 = nc.dram_tensor("tsc", [TT, n_sc, 128, KT, 128], sdt,
                              kind="Internal").ap()

        def scr_block(i, tt):
            return scr0[tt, i]

    with tile.TileContext(nc) as tc, ExitStack() as ctx:
        const = ctx.enter_context(tc.tile_pool(name="const", bufs=1))
        psum = ctx.enter_context(tc.tile_pool(name="psum", bufs=2, space="PSUM"))

        ident = const.tile([128, 128], F32, tag="ident")
        masks.make_identity(nc, ident[:])

        wo_sb = const.tile([128, KT, D], F32, tag="wo")
        wt_sb = const.tile([128, KT, D], F32, tag="wt")
        nc.sync.dma_start(wo_sb[:], w_on.rearrange("(k p) n -> p k n", p=128))
        nc.sync.dma_start(wt_sb[:], w_tg.rearrange("(k p) n -> p k n", p=128))

        # student stationary operands for the similarity matmul
        if b3:
            yt_hi = const.tile([128, KT, RPC], BF16, tag="yt_hi")
            yt_lo = const.tile([128, KT, RPC], BF16, tag="yt_lo")
            if X8:
                yt_hi16 = const.tile([128, KT, RPC], BF16, tag="yt_hi16")
                ys_f8 = const.tile([128, KT, 2, RPC], F8E5, tag="ys_f8")
        else:
            yt_f = const.tile([128, KT, RPC], F32, tag="yt_f")

        def ps_tile(tag):
            return psum.tile([128, 512], F32, tag=tag, name=tag)

        def encode_tile(ab, x_rows, w_sb):
            """x rows [128, D] --DMA+transpose+matmul--> psum tile holding y."""
            xt_in = ab.tile([128, D], F32, tag="x_in")
            nc.sync.dma_start(xt_in[:], x_rows)
            xT = ab.tile([128, KT, 128], F32, tag="xT")
            _transpose_512(nc, psum, xT[:], xt_in[:], ident[:], tag="ps_tx")
            psy = ps_tile("ps_y")
            for k in range(KT):
                nc.tensor.matmul(psy[:], xT[:, k, :], w_sb[:, k, :],
                                 start=(k == 0), stop=(k == KT - 1))
            return psy

        def teacher_norm_tile(ab, psy, dest_fn):
            """psum y -> normalized+transposed (+hi/lo split) -> dest_fn(i) DRAM."""
            sq = ab.tile([128, D], F32, tag="sq")
            n2 = ab.tile([128, 1], F32, tag="n2")
            nc.scalar.activation(sq[:], psy[:], AF.Square, accum_out=n2[:])
            inv = ab.tile([128, 1], F32, tag="inv")
            nc.vector.reciprocal(inv[:], n2[:])
            rn = ab.tile([128, 1], F32, tag="rn")
            nc.scalar.activation(rn[:], inv[:], AF.Sqrt)
            tn = ab.tile([128, D], F32, tag="tn")
            nc.vector.tensor_scalar_mul(tn[:], psy[:], rn[:])
            tnT = ab.tile([128, KT, 128], F32, tag="tnT")
            _transpose_512(nc, psum, tnT[:], tn[:], ident[:], tag="ps_tn")
            if b3:
                tnh = ab.tile([128, KT, 128], BF16, tag="tnh")
                tnl = ab.tile([128, KT, 128], BF16, tag="tnl")
                tmp = ab.tile([128, KT, 128], F32, tag="tmp")
                nc.vector.tensor_copy(tnh[:], tnT[:])
                nc.vector.tensor_sub(tmp[:], tnT[:], tnh[:])
                nc.vector.tensor_copy(tnl[:], tmp[:])
                nc.sync.dma_start(dest_fn(0), tnh[:])
                nc.sync.dma_start(dest_fn(1), tnl[:])
            else:
                nc.sync.dma_start(dest_fn(0), tnT[:])

        def emit_phase_ab(ab):
            if cc:
                # Phase B_own: normalize own teacher slice + emit target_y.
                # 1-tile software pipeline: emit tile t+1's encode (PE work)
                # before tile t's normalization chain (ACT/DVE latency) so the
                # PE never waits on the norm chain.
                def finish_teacher(t, psy):
                    yb = ab.tile([128, D], F32, tag="ya", name="yb")
                    nc.scalar.copy(yb[:], psy[:])
                    nc.sync.dma_start(ty[bass.ts(t, 128), :], yb[:])
                    teacher_norm_tile(ab, psy, lambda i, t=t: loc_block(i, t))

                prev = None
                for t in range(ST):
                    psy = encode_tile(ab, tx_own[bass.ts(t, 128), :], wt_sb)
                    if prev is not None:
                        finish_teacher(*prev)
                    prev = (t, psy)
                finish_teacher(*prev)
                if cc_emit:
                    for gth in range(NGATH):
                        nc.gpsimd.collective_compute(
                            "AllGather",
                            mybir.AluOpType.bypass,
                            replica_groups=[list(range(NCORES))],
                            ins=[loc[bass.ts(gth, TPG)]],
                            outs=[scr[gth]],
                        )
            else:
                # Phase B: recompute the full teacher on every core
                for tt in range(TT):
                    psy = encode_tile(ab, tx_full[bass.ts(tt, 128), :], wt_sb)
                    teacher_norm_tile(ab, psy, lambda i, tt=tt: scr_block(i, tt))
                # Phase B': target own rows -> target_y output
                for t in range(ST):
                    psy = encode_tile(ab, tx_own[bass.ts(t, 128), :], wt_sb)
                    yb = ab.tile([128, D], F32, tag="ya", name="yb")
                    nc.scalar.copy(yb[:], psy[:])
                    nc.sync.dma_start(ty[bass.ts(t, 128), :], yb[:])

            # Phase A: student own rows -> online_y + yT stationary
            for t in range(ST):
                psy = encode_tile(ab, ox[bass.ts(t, 128), :], wo_sb)
                ya = ab.tile([128, D], F32, tag="ya", name="ya")
                nc.scalar.copy(ya[:], psy[:])
                nc.sync.dma_start(oy[bass.ts(t, 128), :], ya[:])
                yT = ab.tile([128, KT, 128], F32, tag="yT", name="yT")
                _transpose_512(nc, psum, yT[:], ya[:], ident[:], tag="ps_tn")
                if b3:
                    nc.vector.tensor_copy(yt_hi[:, :, bass.ts(t, 128)], yT[:])
                    tmp2 = ab.tile([128, KT, 128], F32, tag="tmp2", name="tmp2")
                    nc.vector.tensor_sub(tmp2[:], yT[:], yt_hi[:, :, bass.ts(t, 128)])
                    nc.vector.tensor_copy(yt_lo[:, :, bass.ts(t, 128)], tmp2[:])
                    if X8:
                        nc.vector.tensor_scalar_mul(
                            yt_hi16[:, :, bass.ts(t, 128)],
                            yt_hi[:, :, bass.ts(t, 128)], 16.0)
                        nc.vector.tensor_scalar_mul(
                            ys_f8[:, :, 0, bass.ts(t, 128)],
                            yt_hi[:, :, bass.ts(t, 128)], 1.0 / 16.0)
                        nc.vector.tensor_scalar_mul(
                            ys_f8[:, :, 1, bass.ts(t, 128)], tmp2[:], 16.0)
                else:
                    nc.vector.tensor_copy(yt_f[:, :, bass.ts(t, 128)], yT[:])

        def emit_phase_c(cstream, csegs, ccand):
            _phase_c_body(cstream, csegs, ccand)

        def _phase_c_body(cstream, csegs, ccand):
            for sweep in range(NSWEEP):
                _phase_c_sweep(cstream, csegs, ccand, sweep)

        loop_n = int(os.environ.get("BGRL_LOOP", "0"))
        trunc = int(os.environ.get("BGRL_TRUNC", "0"))

        def _phase_c_sweep(cstream, csegs, ccand, sweep):
            segs = [csegs.tile([128, SEG], F32, tag=f"seg{st}", name=f"seg{st}")
                    for st in range(G)]
            cvs = [ccand.tile([128, NSEG, 8], F32, tag=f"cv{st}", name=f"cv{st}")
                   for st in range(G)]
            cis = [ccand.tile([128, NSEG, 8], U32, tag=f"ci{st}", name=f"ci{st}")
                   for st in range(G)]
            for c in range(trunc if trunc else NCHUNK):
                tstr = []
                for i in range(n_sc):
                    tsb = cstream.tile([128, KT, 4, 128], sdt, tag=f"ts{i}",
                                       name=f"ts{i}")
                    for j in range(4):
                        nc.sync.dma_start(tsb[:, :, j, :], scr_block(i, 4 * c + j))
                    tstr.append(tsb)
                if b3 and X8:
                    tf8 = cstream.tile([128, KT, 2, 4, 128], F8E5, tag="tf8",
                                       name="tf8")
                    nc.vector.tensor_scalar_mul(tf8[:, :, 0, :, :], tstr[1][:], 256.0)
                    nc.vector.tensor_scalar_mul(tf8[:, :, 1, :, :], tstr[0][:], 1.0)
                for st in range(G):
                    g = sweep * G + st
                    ps = ps_tile("ps_c")
                    if b3 and X8:
                        th = tstr[0]
                        for k in range(KT):
                            nc.tensor.matmul(ps[:], yt_hi16[:, k, bass.ts(g, 128)],
                                             th[:, k, :, :],
                                             start=(k == 0), stop=False)
                        for k in range(KT):
                            nc.tensor.matmul(
                                ps[:], ys_f8[:, k, :, bass.ts(g, 128)],
                                tf8[:, k, :, :, :],
                                perf_mode=mybir.MatmulPerfMode.DoubleRow,
                                start=False, stop=(k == KT - 1))
                    elif b3:
                        th, tl = tstr
                        n_mm = 3 * KT
                        i = 0
                        for k in range(KT):
                            s_hi = yt_hi[:, k, bass.ts(g, 128)]
                            s_lo = yt_lo[:, k, bass.ts(g, 128)]
                            for lhs, rhs in ((s_hi, th), (s_hi, tl), (s_lo, th)):
                                nc.tensor.matmul(ps[:], lhs, rhs[:, k, :, :],
                                                 start=(i == 0), stop=(i == n_mm - 1))
                                i += 1
                    else:
                        for k in range(KT):
                            nc.tensor.matmul(ps[:], yt_f[:, k, bass.ts(g, 128)],
                                             tstr[0][:, k, :, :],
                                             start=(k == 0), stop=(k == KT - 1))
                    nc.scalar.copy(segs[st][:, bass.ts(c % CPS, CH)], ps[:])
                if c % CPS == CPS - 1:
                    si = c // CPS
                    for st in range(G):
                        nc.vector.max(cvs[st][:, si, :], segs[st][:])
                        nc.vector.max_index(cis[st][:, si, :], cvs[st][:, si, :],
                                            segs[st][:])
            for st in range(G):
                g = sweep * G + st
                nc.sync.dma_start(cv[bass.ts(g, 128)], cvs[st][:])
                nc.sync.dma_start(ci[bass.ts(g, 128)], cis[st][:])

        if loop_n:
            # timing mode: run the whole kernel body in a hardware loop
            ab = ctx.enter_context(tc.tile_pool(name="ab", bufs=2))
            cstream = ctx.enter_context(tc.tile_pool(name="cstream", bufs=3))
            csegs = ctx.enter_context(tc.tile_pool(name="csegs", bufs=1))
            ccand = ctx.enter_context(tc.tile_pool(name="ccand", bufs=2))
            with tc.For_i(0, loop_n, 1):
                emit_phase_ab(ab)
                emit_phase_c(cstream, csegs, ccand)
        else:
            with tc.tile_pool(name="ab", bufs=3) as ab:
                emit_phase_ab(ab)
            cstream = ctx.enter_context(tc.tile_pool(name="cstream", bufs=3))
            csegs = ctx.enter_context(tc.tile_pool(name="csegs", bufs=1))
            ccand = ctx.enter_context(tc.tile_pool(name="ccand", bufs=2))
            emit_phase_c(cstream, csegs, ccand)

    nc.compile()
    return nc


def _get_nc(mode):
    if mode not in _CACHE:
        _CACHE[mode] = build(mode)
    return _CACHE[mode]


def _host_topk_rows(online_y, target_y, rows):
    """Exact host recompute of top-8 teacher indices for the given rows."""
    tn = target_y / np.maximum(
        np.sqrt((target_y.astype(np.float64) ** 2).sum(-1, keepdims=True)), 1e-12)
    s = online_y[rows].astype(np.float64)
    sim = s @ tn.T.astype(np.float64)
    return np.argsort(-sim, axis=1, kind="stable")[:, :8]


def kernel(online_x, target_x, W_online, W_pred, W_target, k):
    global LAST_EXEC_NS, LAST_RESULTS
    assert int(k) == 8
    online_x = np.ascontiguousarray(np.asarray(online_x, dtype=np.float32))
    target_x = np.ascontiguousarray(np.asarray(target_x, dtype=np.float32))
    W_online = np.ascontiguousarray(np.asarray(W_online, dtype=np.float32))
    W_target = np.ascontiguousarray(np.asarray(W_target, dtype=np.float32))

    def _run(mode):
        nc = _get_nc(mode)
        cc = mode.endswith("cc")
        in_maps = []
        for c in range(NCORES):
            sl = slice(c * RPC, (c + 1) * RPC)
            m = {
                "online_x_own": np.ascontiguousarray(online_x[sl]),
                "target_x_own": np.ascontiguousarray(target_x[sl]),
                "W_online": W_online,
                "W_target": W_target,
            }
            if not cc:
                m["target_x_full"] = target_x
            in_maps.append(m)
        trace = os.environ.get("BGRL_TRACE", "0") == "1"
        return run_bass_kernel_spmd(nc, in_maps, core_ids=list(range(NCORES)),
                                    trace=trace)

    try:
        res = _run(MODE)
    except Exception:
        if not MODE.endswith("cc"):
            raise
        # collective path failed in this environment: fall back to the
        # variant that recomputes the full teacher on every core
        res = _run(MODE[:-2])
    LAST_EXEC_NS = res.exec_time_ns
    LAST_RESULTS = res

    outs = res.results
    online_y = np.concatenate([outs[c]["online_y"] for c in range(NCORES)], axis=0)
    target_y = np.concatenate([outs[c]["target_y"] for c in range(NCORES)], axis=0)
    vals = np.concatenate([outs[c]["cand_v"] for c in range(NCORES)], axis=0)
    idxs = np.concatenate([outs[c]["cand_i"] for c in range(NCORES)], axis=0)

    # merge the NSEG per-segment candidate lists
    gi = idxs.astype(np.int64) + (np.arange(NSEG, dtype=np.int64) * SEG)[None, :, None]
    flat_i = gi.reshape(N, NSEG * 8)

    # defensive: MaxIndex emits 0xFFFFFFFF when a value is unmatched; clamp those
    # candidates to a valid index (their value row is recomputed below anyway)
    bad_mask = (flat_i < 0) | (flat_i >= N)
    bad_rows = np.unique(np.nonzero(bad_mask)[0])
    flat_i[bad_mask] = 0

    if os.environ.get("BGRL_RESCORE", "1") == "1":
        # rescore the 64 candidates per row on host in fp32 (cheap, exact
        # ordering match with the fp32 reference up to its own rounding)
        def l2n(x):
            nn = np.sqrt((x * x).sum(-1, keepdims=True))
            return x / np.maximum(nn, 1e-12)

        student = l2n(online_y)
        teacher = l2n(target_y)
        top_idx = np.zeros((N, 8), dtype=np.int64)
        B = 2048
        for i in range(0, N, B):
            cand = np.sort(flat_i[i:i + B], axis=1)      # index-ascending
            tvec = teacher[cand]                          # [B, 64, 512]
            sims = np.matmul(tvec, student[i:i + B, :, None])[:, :, 0]  # [B, 64]
            o2 = np.argsort(-sims, axis=1, kind="stable")[:, :8]
            top_idx[i:i + B] = np.take_along_axis(cand, o2, axis=1)
    else:
        flat_v = vals.reshape(N, NSEG * 8)
        order = np.argsort(-flat_v, axis=1, kind="stable")[:, :8]
        top_idx = np.take_along_axis(flat_i, order, axis=1)

    if bad_rows.size:
        top_idx[bad_rows] = _host_topk_rows(online_y, target_y, bad_rows)

    rows = np.repeat(np.arange(N, dtype=np.int32), 8)
    cols = top_idx.reshape(-1).astype(np.int32)
    knn = np.stack([rows, cols], axis=0)
    return online_y, target_y, knn
